# revision 10
# baseline (speedup 1.0000x reference)
"""Trainium2 Bass kernel for the attention+LN+MLP block (nn_Attention_84310208020626).

Reference computation (per batch b):
    q = x_b @ Wq.T ; k = x_b @ Wk.T ; v = x_b @ Wv.T          (S=2048, D=512)
    attn = softmax(q k^T / sqrt(512))
    res  = attn @ v
    h    = LayerNorm(res) * ln_g + ln_b
    out  = relu(h @ W1.T + b1) @ W2.T + b2

Sharding: 8 cores = 4 batches x 2 sequence halves. Every core computes its
batch's full K/V (recompute, no collectives) and runs attention + LN + MLP
for its own 1024 query rows.

Device layout: activations are feature-major [feature, seq] so that every
GEMM contracts over the partition dimension without transposes:
    GT[d',s]     = A-stationary GEMM over xT, A = Wq^T Wk precomputed on host
                   (scores = q k^T = (x A) x^T, so no separate Q/K GEMMs)
    V[t,e]       = xT-stationary GEMM (t-major, feeds the res GEMM as lhsT)
    scoresT[t,s] = xT-stationary GEMM, rhs = GT     -> exp -> expT (bf16)
    resU[e,s]    = V-stationary GEMM, rhs = expT  (softmax denom NOT applied)
    sums[1,s]    = ones-stationary GEMM over expT
LayerNorm over e (partition dim) uses ones-matmul column sums; the softmax
division is folded into LN via scale invariance with a corrected epsilon:
    LN(res) = (resU - muU) / sqrt(varU + eps*sums^2)  (exact in exact arithmetic)
and the whole LN is folded into the MLP1 GEMM epilogue:
    h1 = relu( (G1 @ res)*rstd[s] - murstd[s]*r1[f] + (W1@ln_b)[f] + b1[f] )
with G1 = W1*diag(ln_g), r1 = G1 row sums (both computed on device once).
Per-column stats are broadcast across partitions with a K=1 ones matmul.
All GEMM operands are bf16 (fp32 PSUM accumulation); LN stats math is fp32.
"""

import ml_dtypes
import numpy as np

import concourse.bass as bass
import concourse.mybir as mybir
import concourse.tile as tile
from concourse import bacc
from concourse.bass_utils import run_bass_kernel_spmd

S, B, D = 2048, 4, 512
N_CORES = 8
SQ = 1024          # query rows per core
SBLK = 512         # s-block (pipeline granularity)
NBLK = SQ // SBLK  # 2
ND = D // 128      # 4 chunks of the feature dims
NT = S // 128      # 16 t-chunks
NTT = S // 512     # 4 t-tiles of 512 for KT GEMM
EPS = 1e-5
SCALE = 1.0 / float(np.sqrt(512.0))

F32 = mybir.dt.float32
F32R = mybir.dt.float32r
BF16 = mybir.dt.bfloat16
AF = mybir.ActivationFunctionType
ALU = mybir.AluOpType


def _emit(nc, tc, n_iters=1):
    xT = nc.tensor_by_name["xT"].ap()       # (512, 2048) bf16, q-half first
    xTM = nc.tensor_by_name["xTM"].ap()     # (2048, 512) bf16, same t order
    A_qk = nc.tensor_by_name["A_qk"].ap()   # (512, 512) = Wq.T @ Wk  (d, d')
    WvT = nc.tensor_by_name["WvT"].ap()
    W1T = nc.tensor_by_name["W1T"].ap()     # (512, 512) = W1.T  (e, f)
    W2T = nc.tensor_by_name["W2T"].ap()
    b1 = nc.tensor_by_name["b1"].ap()       # (512,)
    b2 = nc.tensor_by_name["b2"].ap()
    ln_g = nc.tensor_by_name["ln_g"].ap()
    ln_b = nc.tensor_by_name["ln_b"].ap()
    outT = nc.tensor_by_name["outT"].ap()   # (512, 1024) fp32 out

    # ---------------- SBUF tiles ----------------
    from contextlib import ExitStack
    ctx = ExitStack()
    consts = ctx.enter_context(tc.tile_pool(name="consts", bufs=1))
    big = ctx.enter_context(tc.tile_pool(name="big", bufs=1))
    qt_pool = ctx.enter_context(tc.tile_pool(name="qt", bufs=2))
    exp_pool = ctx.enter_context(tc.tile_pool(name="expp", bufs=2))
    res_pool = ctx.enter_context(tc.tile_pool(name="resp", bufs=2))
    h1_pool = ctx.enter_context(tc.tile_pool(name="h1p", bufs=2))
    out_pool = ctx.enter_context(tc.tile_pool(name="outp", bufs=2))
    sq_pool = ctx.enter_context(tc.tile_pool(name="sqp", bufs=4))
    row_pool = ctx.enter_context(tc.tile_pool(name="rowp", bufs=2))
    bc_pool = ctx.enter_context(tc.tile_pool(name="bcp", bufs=2))

    mm_psum = ctx.enter_context(tc.tile_pool(name="mmps", bufs=8, space="PSUM"))

    # constants / weights
    a_sb = consts.tile([128, ND, D], BF16)    # (p, dc, d')
    wv_sb = consts.tile([128, ND, D], BF16)
    w1_sb = consts.tile([128, ND, D], BF16)
    w2_sb = consts.tile([128, ND, D], BF16)
    b1_sb = consts.tile([128, ND], F32)
    b2_sb = consts.tile([128, ND], F32)
    g_sb = consts.tile([128, ND], F32)
    lb_sb = consts.tile([128, ND], F32)
    # A (needed first, for GT) gets its own queue; everything else later.
    ar = A_qk.rearrange("(dc p) e -> p dc e", p=128)
    nc.gpsimd.dma_start(out=a_sb[:, :, :], in_=ar[:, :, :])
    for v_sb, v_dram in ((b1_sb, b1), (b2_sb, b2), (g_sb, ln_g), (lb_sb, ln_b)):
        nc.gpsimd.dma_start(out=v_sb[:, :],
                            in_=v_dram.rearrange("(c p) -> p c", p=128))
    wvr = WvT.rearrange("(dc p) e -> p dc e", p=128)
    nc.scalar.dma_start(out=wv_sb[:, :, :], in_=wvr[:, :, :])
    for w_sb, w_dram in ((w1_sb, W1T), (w2_sb, W2T)):
        wr = w_dram.rearrange("(dc p) e -> p dc e", p=128)
        nc.gpsimd.dma_start(out=w_sb[:, :, :], in_=wr[:, :, :])

    gb_sb = consts.tile([128, ND, 2], BF16)
    rw_sb = consts.tile([128, ND, 2], F32)
    w1bb1_sb = consts.tile([128, ND], F32)

    nc.vector.tensor_copy(out=gb_sb[:, :, 0], in_=g_sb[:, :])
    nc.vector.tensor_copy(out=gb_sb[:, :, 1], in_=lb_sb[:, :])

    def emit_ln_fold_precompute():
        # r1[f] = sum_e W1[f,e] g[e],  w1b[f] = sum_e W1[f,e] ln_b[e]
        # then G1 = W1 * g[e] in place (folds LayerNorm into the MLP1 GEMM)
        for fc in range(ND):
            rps1 = mm_psum.tile([128, 2], F32, tag="mm")
            for ec in range(ND):
                nc.tensor.matmul(
                    rps1[:, :],
                    w1_sb[:, ec, fc * 128:(fc + 1) * 128],
                    gb_sb[:, ec, :],
                    start=(ec == 0), stop=(ec == ND - 1),
                )
            nc.vector.tensor_copy(out=rw_sb[:, fc, :], in_=rps1[:, :])
        nc.vector.tensor_add(out=w1bb1_sb[:, :], in0=rw_sb[:, :, 1], in1=b1_sb[:, :])
        for ec in range(ND):
            nc.vector.tensor_scalar_mul(out=w1_sb[:, ec, :], in0=w1_sb[:, ec, :],
                                        scalar1=g_sb[:, ec:ec + 1])

    ones128 = nc.tensor_by_name["ones128"].ap()  # (128,) of 1.0
    ones_col_b = consts.tile([128, 1], BF16)   # stationary for column sums
    nc.vector.memset(ones_col_b, 1.0)
    ones_row = consts.tile([1, 128], F32R)      # stationary for partition broadcast
    nc.sync.dma_start(out=ones_row[:, :],
                      in_=ones128.bitcast(F32R).rearrange("(c p) -> c p", c=1))

    for _iter in range(n_iters):
        _emit_iter(nc, tc, xT, xTM, outT, big, qt_pool, exp_pool, res_pool, h1_pool,
                   out_pool, sq_pool, row_pool, bc_pool, mm_psum,
                   a_sb, wv_sb, w1_sb, w2_sb, b1_sb, b2_sb, g_sb, lb_sb,
                   ones_col_b, ones_row, rw_sb, w1bb1_sb,
                   emit_ln_fold_precompute if _iter == 0 else None)

    ctx.close()


def _emit_iter(nc, tc, xT, xTM, outT, big, qt_pool, exp_pool, res_pool, h1_pool,
               out_pool, sq_pool, row_pool, bc_pool, mm_psum,
               a_sb, wv_sb, w1_sb, w2_sb, b1_sb, b2_sb, g_sb, lb_sb,
               ones_col_b, ones_row, rw_sb, w1bb1_sb,
               precompute=None):
    # x (feature-major), query half occupies columns [0, 1024)
    # First tile (needed by the GT GEMM immediately): one dc chunk per DMA
    # queue so all four transfer in parallel and PE can start ASAP.
    x_sb = big.tile([128, ND, S], BF16, tag="x", name="x_sb")
    xr = xT.rearrange("(dc p) t -> p dc t", p=128)
    q_engs = (nc.sync, nc.scalar, nc.sync, nc.scalar)
    for dc in range(ND):
        q_engs[dc].dma_start(out=x_sb[:, dc, 0:512], in_=xr[:, dc, 0:512])
    # Remaining t-tiles (needed by the scores GEMM): one tile per queue.
    for tt in range(1, NTT):
        q_engs[tt - 1].dma_start(out=x_sb[:, :, tt * 512:(tt + 1) * 512],
                                 in_=xr[:, :, tt * 512:(tt + 1) * 512])

    # x in t-major layout: stationary of the Z = x^T @ exp GEMM
    xtm_sb = big.tile([128, NT, D], BF16, tag="v", name="xtm_sb")
    xmr = xTM.rearrange("(tc p) d -> p tc d", p=128)
    for g in range(4):
        q_engs[g].dma_start(out=xtm_sb[:, 4 * g:4 * (g + 1), :],
                            in_=xmr[:, 4 * g:4 * (g + 1), :])

    # ------- GT = A-stationary GEMM (G = x @ A; scores = G @ x^T) -------
    qt_tiles = []
    for sb in range(NBLK):
        s0 = sb * SBLK
        qt_sb = qt_pool.tile([128, ND, SBLK], BF16, tag="qt")
        for ec in range(ND):
            qps = mm_psum.tile([128, 512], F32, tag="mm")
            for dc in range(ND):
                nc.tensor.matmul(
                    qps[:, :],
                    a_sb[:, dc, ec * 128:(ec + 1) * 128],
                    x_sb[:, dc, s0:s0 + SBLK],
                    start=(dc == 0), stop=(dc == ND - 1),
                )
            nc.scalar.copy(out=qt_sb[:, ec, :], in_=qps[:, :])
        qt_tiles.append(qt_sb)

    # ---------------- per s-block pipeline (software-pipelined emission) ----
    # emission order: scores(0), res(0), scores(1), stats(0), res(1),
    # norm+mlp(0), stats(1), norm+mlp(1) - keeps matmul work queued on PE
    # while DVE/ACT compute the LN row stats of the previous block.
    exp_tiles = [None] * NBLK
    res_tiles = [None] * NBLK
    rows2_tiles = [None] * NBLK

    def emit_scores(sb):
        qt_sb = qt_tiles[sb]
        exp_sb = exp_pool.tile([128, NT, SBLK], BF16, tag="exp", name=f"exp{sb}")
        for tc_i in range(NT):
            sps = mm_psum.tile([128, 512], F32, tag="mm")
            for dc in range(ND):
                nc.tensor.matmul(
                    sps[:, :],
                    x_sb[:, dc, tc_i * 128:(tc_i + 1) * 128],
                    qt_sb[:, dc, :],
                    start=(dc == 0), stop=(dc == ND - 1),
                )
            nc.scalar.activation(out=exp_sb[:, tc_i, :], in_=sps[:, :],
                                 func=AF.Exp, scale=SCALE)
        exp_tiles[sb] = exp_sb

    def emit_res(sb):
        exp_sb = exp_tiles[sb]
        # Z[d, s] = sum_t x[t,d] * exp[t,s]   (x t-major stationary)
        z_sb = sq_pool.tile([128, ND, SBLK], BF16, tag="z", name=f"z{sb}")
        for dc in range(ND):
            zps = mm_psum.tile([128, 512], F32, tag="mm")
            for tc_i in range(NT):
                nc.tensor.matmul(
                    zps[:, :],
                    xtm_sb[:, tc_i, dc * 128:(dc + 1) * 128],
                    exp_sb[:, tc_i, :],
                    start=(tc_i == 0), stop=(tc_i == NT - 1),
                )
            nc.scalar.copy(out=z_sb[:, dc, :], in_=zps[:, :])
        # resU[e, s] = Wv @ Z
        res_sb = res_pool.tile([128, ND, SBLK], BF16, tag="res", name=f"res{sb}")
        for ec in range(ND):
            rps = mm_psum.tile([128, 512], F32, tag="mm")
            for dc in range(ND):
                nc.tensor.matmul(
                    rps[:, :],
                    wv_sb[:, dc, ec * 128:(ec + 1) * 128],
                    z_sb[:, dc, :],
                    start=(dc == 0), stop=(dc == ND - 1),
                )
            nc.vector.tensor_copy(out=res_sb[:, ec, :], in_=rps[:, :])
        res_tiles[sb] = res_sb

    def emit_stats(sb):
        exp_sb = exp_tiles[sb]
        res_sb = res_tiles[sb]
        sums_ps = mm_psum.tile([1, 512], F32, tag="mm")
        for tc_i in range(NT):
            nc.tensor.matmul(
                sums_ps[:, :], ones_col_b[:, :], exp_sb[:, tc_i, :],
                start=(tc_i == 0), stop=(tc_i == NT - 1),
            )
        sume_ps = mm_psum.tile([1, 512], F32, tag="mm")
        for ec in range(ND):
            nc.tensor.matmul(
                sume_ps[:, :], ones_col_b[:, :],
                res_sb[:, ec, :],
                start=(ec == 0), stop=(ec == ND - 1),
            )
        sumsq_ps = mm_psum.tile([1, 512], F32, tag="mm")
        for ec in range(ND):
            sq_sb = sq_pool.tile([128, SBLK], BF16, tag="sq")
            nc.vector.tensor_mul(out=sq_sb[:, :], in0=res_sb[:, ec, :],
                                 in1=res_sb[:, ec, :])
            nc.tensor.matmul(
                sumsq_ps[:, :], ones_col_b[:, :],
                sq_sb[:, :],
                start=(ec == 0), stop=(ec == ND - 1),
            )

        # row stats on one partition:
        #   muU = sumE/512 ; varU = sumSq/512 - muU^2
        #   rstd = 1/sqrt(varU + EPS*sums^2) ; murstd = muU*rstd
        rows = row_pool.tile([1, 4, SBLK], F32, tag="rows", name=f"rows{sb}")
        rows2 = row_pool.tile([1, 2, SBLK], F32R, tag="rows2", name=f"rows2{sb}")
        nc.scalar.mul(out=rows[:, 0, :], in_=sume_ps[:, :], mul=-1.0 / D)    # -muU
        nc.scalar.mul(out=rows[:, 1, :], in_=sumsq_ps[:, :], mul=1.0 / D)    # msq
        nc.scalar.activation(out=rows[:, 2, :], in_=sums_ps[:, :],
                             func=AF.Square, scale=float(np.sqrt(EPS)))      # eps*sums^2
        nc.vector.tensor_mul(out=rows[:, 3, :], in0=rows[:, 0, :], in1=rows[:, 0, :])
        nc.vector.tensor_sub(out=rows[:, 1, :], in0=rows[:, 1, :], in1=rows[:, 3, :])
        nc.vector.tensor_add(out=rows[:, 1, :], in0=rows[:, 1, :], in1=rows[:, 2, :])
        nc.scalar.activation(out=rows[:, 1, :], in_=rows[:, 1, :], func=AF.Sqrt)
        nc.vector.reciprocal_approx_fast(out=rows[:, 3, :], in_=rows[:, 1, :])
        nc.vector.tensor_copy(out=rows2[:, 0, :], in_=rows[:, 3, :])         # rstd
        nc.vector.tensor_mul(out=rows2[:, 1, :], in0=rows[:, 0, :],
                             in1=rows[:, 3, :])                              # -murstd
        rows2_tiles[sb] = rows2

    p_tiles = [None] * NBLK
    h1_tiles = [None] * NBLK

    def emit_p(sb):
        # P = G1 @ res (independent of the LN stats chain)
        res_sb = res_tiles[sb]
        p_ps = []
        for fc in range(ND):
            hps = mm_psum.tile([128, 512], F32, tag="mm", name=f"p{sb}_{fc}")
            for ec in range(ND):
                nc.tensor.matmul(
                    hps[:, :],
                    w1_sb[:, ec, fc * 128:(fc + 1) * 128],
                    res_sb[:, ec, :],
                    start=(ec == 0), stop=(ec == ND - 1),
                )
            p_ps.append(hps)
        p_tiles[sb] = p_ps

    def emit_bc_epi(sb):
        rows2 = rows2_tiles[sb]
        p_ps = p_tiles[sb]

        # broadcast [rstd; -murstd] across 128 partitions via K=1 matmul
        bc_sb = bc_pool.tile([128, 2, SBLK], F32, tag="bc_sb")
        for j in range(2):
            bc_ps = mm_psum.tile([128, 512], F32, tag="mm")
            nc.tensor.matmul(
                bc_ps[:, :], ones_row[:, :],
                rows2[:, j, :], start=True, stop=True,
            )
            nc.scalar.copy(out=bc_sb[:, j, :], in_=bc_ps[:, :])

        # fused MLP1 + LayerNorm epilogue:
        #   h1 = relu( P*rstd[s] - murstd[s]*r1[f] + w1b[f] + b1[f] )
        h1_sb = h1_pool.tile([128, ND, SBLK], BF16, tag="h1", name=f"h1_{sb}")
        for fc in range(ND):
            t_sb = sq_pool.tile([128, SBLK], F32R, tag="sq")
            nc.vector.tensor_mul(out=t_sb[:, :], in0=p_ps[fc][:, :],
                                 in1=bc_sb[:, 0, :])
            nc.vector.scalar_tensor_tensor(
                out=t_sb[:, :], in0=bc_sb[:, 1, :],
                scalar=rw_sb[:, fc, 0:1], in1=t_sb[:, :],
                op0=ALU.mult, op1=ALU.add,
            )
            nc.scalar.activation(out=h1_sb[:, fc, :], in_=t_sb[:, :],
                                 func=AF.Relu, bias=w1bb1_sb[:, fc:fc + 1])
        h1_tiles[sb] = h1_sb

    def emit_mlp2(sb):
        s0 = sb * SBLK
        h1_sb = h1_tiles[sb]
        o_sb = out_pool.tile([128, ND, SBLK], BF16, tag="o")
        outr = outT[:, s0:s0 + SBLK].rearrange("(gc p) s -> p gc s", p=128)
        for gc in range(ND):
            ops = mm_psum.tile([128, 512], F32, tag="mm")
            for fc in range(ND):
                nc.tensor.matmul(
                    ops[:, :],
                    w2_sb[:, fc, gc * 128:(gc + 1) * 128],
                    h1_sb[:, fc, :],
                    start=(fc == 0), stop=(fc == ND - 1),
                )
            nc.scalar.activation(out=o_sb[:, gc, :], in_=ops[:, :],
                                 func=AF.Identity, bias=b2_sb[:, gc:gc + 1])
            eng = (nc.sync, nc.scalar, nc.gpsimd, nc.sync)[gc]
            eng.dma_start(out=outr[:, gc, :], in_=o_sb[:, gc, :])

    emit_scores(0)
    emit_res(0)
    emit_stats(0)
    if precompute is not None:
        precompute()
    emit_scores(1)
    emit_res(1)
    emit_stats(1)
    emit_p(0)
    emit_bc_epi(0)
    emit_mlp2(0)
    emit_p(1)
    emit_bc_epi(1)
    emit_mlp2(1)


def build_nc(n_iters=1):
    nc = bacc.Bacc("TRN2", target_bir_lowering=False, debug=False)
    nc.tensor_by_name = {}

    def dram(name, shape, kind):
        t = nc.dram_tensor(name, shape, F32, kind=kind)
        nc.tensor_by_name[name] = t
        return t

    def dram_bf(name, shape, kind):
        t = nc.dram_tensor(name, shape, BF16, kind=kind)
        nc.tensor_by_name[name] = t
        return t

    dram_bf("xT", [D, S], "ExternalInput")
    dram_bf("xTM", [S, D], "ExternalInput")
    for nm in ("A_qk", "WvT", "W1T", "W2T"):
        dram_bf(nm, [D, D], "ExternalInput")
    for nm in ("b1", "b2", "ln_g", "ln_b"):
        dram(nm, [D], "ExternalInput")
    dram("ones128", [128], "ExternalInput")
    dram_bf("outT", [D, SQ], "ExternalOutput")

    with tile.TileContext(nc) as tc:
        _emit(nc, tc, n_iters=n_iters)
    nc.compile()
    return nc


_CACHED_NC = None


def _get_nc():
    global _CACHED_NC
    if _CACHED_NC is None:
        _CACHED_NC = build_nc()
    return _CACHED_NC


def make_in_maps(x, Wq, Wk, Wv, ln_g, ln_b, W1, b1, W2, b2):
    BF = ml_dtypes.bfloat16
    x = np.asarray(x, dtype=np.float32)
    A_qk = np.asarray(Wq, np.float32).T @ np.asarray(Wk, np.float32)
    shared = {
        "A_qk": np.ascontiguousarray(A_qk.astype(BF)),
        "WvT": np.ascontiguousarray(np.asarray(Wv, np.float32).T.astype(BF)),
        "W1T": np.ascontiguousarray(np.asarray(W1, np.float32).T.astype(BF)),
        "W2T": np.ascontiguousarray(np.asarray(W2, np.float32).T.astype(BF)),
        "b1": np.asarray(b1, np.float32),
        "b2": np.asarray(b2, np.float32),
        "ln_g": np.asarray(ln_g, np.float32),
        "ln_b": np.asarray(ln_b, np.float32),
        "ones128": np.ones(128, np.float32),
    }
    in_maps = []
    for c in range(N_CORES):
        b, h = divmod(c, 2)
        xT = x[:, b, :].T  # (512, 2048)
        q = xT[:, h * SQ:(h + 1) * SQ]
        o = xT[:, (1 - h) * SQ:(2 - h) * SQ]
        xp = np.concatenate([q, o], axis=1)  # (512, 2048), q-half first
        in_maps.append({"xT": np.ascontiguousarray(xp.astype(BF)),
                        "xTM": np.ascontiguousarray(xp.T.astype(BF)),
                        **shared})
    return in_maps


def kernel(x, Wq, Wk, Wv, ln_g, ln_b, W1, b1, W2, b2):
    nc = _get_nc()
    in_maps = make_in_maps(x, Wq, Wk, Wv, ln_g, ln_b, W1, b1, W2, b2)
    res = run_bass_kernel_spmd(nc, in_maps, list(range(N_CORES)))
    out = np.empty((S, B, D), dtype=np.float32)
    for c in range(N_CORES):
        b, h = divmod(c, 2)
        out[h * SQ:(h + 1) * SQ, b, :] = res.results[c]["outT"].T.astype(np.float32)
    return out



# revision 21
# speedup vs baseline: 1.0762x; 1.0762x over previous
"""Trainium2 Bass kernel for the attention+LN+MLP block (nn_Attention_84310208020626).

Reference computation (per batch b):
    q = x_b @ Wq.T ; k = x_b @ Wk.T ; v = x_b @ Wv.T          (S=2048, D=512)
    attn = softmax(q k^T / sqrt(512))
    res  = attn @ v
    h    = LayerNorm(res) * ln_g + ln_b
    out  = relu(h @ W1.T + b1) @ W2.T + b2

Sharding: 8 cores = 4 batches x 2 sequence halves. Every core computes its
batch's full K/V (recompute, no collectives) and runs attention + LN + MLP
for its own 1024 query rows.

Device layout: activations are feature-major [feature, seq] so that every
GEMM contracts over the partition dimension without transposes:
    GT[d',s]     = A-stationary GEMM over xT, A = Wq^T Wk precomputed on host
                   (scores = q k^T = (x A) x^T, so no separate Q/K GEMMs)
    V[t,e]       = xT-stationary GEMM (t-major, feeds the res GEMM as lhsT)
    scoresT[t,s] = xT-stationary GEMM, rhs = GT     -> exp -> expT (bf16)
    resU[e,s]    = V-stationary GEMM, rhs = expT  (softmax denom NOT applied)
    sums[1,s]    = ones-stationary GEMM over expT
LayerNorm over e (partition dim) uses ones-matmul column sums; the softmax
division is folded into LN via scale invariance with a corrected epsilon:
    LN(res) = (resU - muU) / sqrt(varU + eps*sums^2)  (exact in exact arithmetic)
and the whole LN is folded into the MLP1 GEMM epilogue:
    h1 = relu( (G1 @ res)*rstd[s] - murstd[s]*r1[f] + (W1@ln_b)[f] + b1[f] )
with G1 = W1*diag(ln_g), r1 = G1 row sums (both computed on device once).
Per-column stats are broadcast across partitions with a K=1 ones matmul.
All GEMM operands are bf16 (fp32 PSUM accumulation); LN stats math is fp32.
"""

import ml_dtypes
import numpy as np

import concourse.bass as bass
import concourse.mybir as mybir
import concourse.tile as tile
from concourse import bacc
from concourse.bass_utils import run_bass_kernel_spmd

S, B, D = 2048, 4, 512
N_CORES = 8
SQ = 1024          # query rows per core
SBLK = 512         # s-block (pipeline granularity)
NBLK = SQ // SBLK  # 2
ND = D // 128      # 4 chunks of the feature dims
NT = S // 128      # 16 t-chunks
NTT = S // 512     # 4 t-tiles of 512 for KT GEMM
EPS = 1e-5
SCALE = 1.0 / float(np.sqrt(512.0))

F32 = mybir.dt.float32
F32R = mybir.dt.float32r
BF16 = mybir.dt.bfloat16
AF = mybir.ActivationFunctionType
ALU = mybir.AluOpType


def _emit(nc, tc, n_iters=1):
    xT = nc.tensor_by_name["xT"].ap()       # (512, 2048) bf16, q-half first
    xTM = nc.tensor_by_name["xTM"].ap()     # (2048, 512) bf16, same t order
    A_qk = nc.tensor_by_name["A_qk"].ap()   # (512, 512) = Wq.T @ Wk  (d, d')
    WvT = nc.tensor_by_name["WvT"].ap()
    W1T = nc.tensor_by_name["W1T"].ap()     # (512, 512) = W1.T  (e, f)
    W2T = nc.tensor_by_name["W2T"].ap()
    b1 = nc.tensor_by_name["b1"].ap()       # (512,)
    b2 = nc.tensor_by_name["b2"].ap()
    ln_g = nc.tensor_by_name["ln_g"].ap()
    ln_b = nc.tensor_by_name["ln_b"].ap()
    outT = nc.tensor_by_name["outT"].ap()   # (512, 1024) fp32 out

    # ---------------- SBUF tiles ----------------
    from contextlib import ExitStack
    ctx = ExitStack()
    consts = ctx.enter_context(tc.tile_pool(name="consts", bufs=1))
    big = ctx.enter_context(tc.tile_pool(name="big", bufs=1))
    qt_pool = ctx.enter_context(tc.tile_pool(name="qt", bufs=2))
    exp_pool = ctx.enter_context(tc.tile_pool(name="expp", bufs=2))
    res_pool = ctx.enter_context(tc.tile_pool(name="resp", bufs=2))
    h1_pool = ctx.enter_context(tc.tile_pool(name="h1p", bufs=2))
    out_pool = ctx.enter_context(tc.tile_pool(name="outp", bufs=2))
    sq_pool = ctx.enter_context(tc.tile_pool(name="sqp", bufs=4))
    row_pool = ctx.enter_context(tc.tile_pool(name="rowp", bufs=2))
    bc_pool = ctx.enter_context(tc.tile_pool(name="bcp", bufs=2))

    mm_psum = ctx.enter_context(tc.tile_pool(name="mmps", bufs=8, space="PSUM"))

    # constants / weights
    a_sb = consts.tile([128, ND, D], BF16)    # (p, dc, d')
    wv_sb = consts.tile([128, ND, D], BF16)
    w1_sb = consts.tile([128, ND, D], BF16)
    w2_sb = consts.tile([128, ND, D], BF16)
    b1_sb = consts.tile([128, ND], F32)
    b2_sb = consts.tile([128, ND], F32)
    g_sb = consts.tile([128, ND], F32)
    lb_sb = consts.tile([128, ND], F32)
    # Input DMAs in need-order, spread over the three DMA queues
    # (sync / scalar / gpsimd) so transfers overlap and the GT GEMM can
    # start as early as possible.
    ar = A_qk.rearrange("(dc p) e -> p dc e", p=128)
    for dc in range(ND):
        nc.scalar.dma_start(out=a_sb[:, dc, :], in_=ar[:, dc, :])

    # x (feature-major), query half occupies columns [0, 1024)
    x_sb = big.tile([128, ND, S], BF16, tag="x", name="x_sb")
    xr = xT.rearrange("(dc p) t -> p dc t", p=128)
    for dc in range(ND):
        nc.sync.dma_start(out=x_sb[:, dc, 0:512], in_=xr[:, dc, 0:512])
    for tt, eng in ((1, nc.sync), (2, nc.scalar), (3, nc.gpsimd)):
        eng.dma_start(out=x_sb[:, :, tt * 512:(tt + 1) * 512],
                      in_=xr[:, :, tt * 512:(tt + 1) * 512])

    # x in t-major layout: stationary of the Z = x^T @ exp GEMM
    xtm_sb = big.tile([128, NT, D], BF16, tag="v", name="xtm_sb")
    xmr = xTM.rearrange("(tc p) d -> p tc d", p=128)
    for g, eng in ((0, nc.sync), (1, nc.scalar), (3, nc.gpsimd), (2, nc.sync)):
        eng.dma_start(out=xtm_sb[:, 4 * g:4 * (g + 1), :],
                      in_=xmr[:, 4 * g:4 * (g + 1), :])

    wvr = WvT.rearrange("(dc p) e -> p dc e", p=128)
    nc.scalar.dma_start(out=wv_sb[:, :, :], in_=wvr[:, :, :])
    for v_sb, v_dram in ((b1_sb, b1), (b2_sb, b2), (g_sb, ln_g), (lb_sb, ln_b)):
        nc.gpsimd.dma_start(out=v_sb[:, :],
                            in_=v_dram.rearrange("(c p) -> p c", p=128))
    wr1 = W1T.rearrange("(dc p) e -> p dc e", p=128)
    nc.gpsimd.dma_start(out=w1_sb[:, :, :], in_=wr1[:, :, :])
    wr2 = W2T.rearrange("(dc p) e -> p dc e", p=128)
    nc.scalar.dma_start(out=w2_sb[:, :, :], in_=wr2[:, :, :])

    gb_sb = consts.tile([128, ND, 2], BF16)
    rw_sb = consts.tile([128, ND, 2], F32)
    w1bb1_sb = consts.tile([128, ND], F32)

    nc.vector.tensor_copy(out=gb_sb[:, :, 0], in_=g_sb[:, :])
    nc.vector.tensor_copy(out=gb_sb[:, :, 1], in_=lb_sb[:, :])

    def emit_ln_fold_precompute():
        # r1[f] = sum_e W1[f,e] g[e],  w1b[f] = sum_e W1[f,e] ln_b[e]
        # then G1 = W1 * g[e] in place (folds LayerNorm into the MLP1 GEMM)
        for fc in range(ND):
            rps1 = mm_psum.tile([128, 2], F32, tag="mm")
            for ec in range(ND):
                nc.tensor.matmul(
                    rps1[:, :],
                    w1_sb[:, ec, fc * 128:(fc + 1) * 128],
                    gb_sb[:, ec, :],
                    start=(ec == 0), stop=(ec == ND - 1),
                )
            nc.vector.tensor_copy(out=rw_sb[:, fc, :], in_=rps1[:, :])
        nc.vector.tensor_add(out=w1bb1_sb[:, :], in0=rw_sb[:, :, 1], in1=b1_sb[:, :])
        for ec in range(ND):
            nc.vector.tensor_scalar_mul(out=w1_sb[:, ec, :], in0=w1_sb[:, ec, :],
                                        scalar1=g_sb[:, ec:ec + 1])

    ones128 = nc.tensor_by_name["ones128"].ap()  # (128,) of 1.0
    ones_col_b = consts.tile([128, 1], BF16)   # stationary for column sums
    nc.vector.memset(ones_col_b, 1.0)
    ones_row = consts.tile([1, 128], F32R)      # stationary for partition broadcast
    nc.gpsimd.dma_start(out=ones_row[:, :],
                        in_=ones128.bitcast(F32R).rearrange("(c p) -> c p", c=1))

    for _iter in range(n_iters):
        _emit_iter(nc, tc, x_sb, xtm_sb, outT, big, qt_pool, exp_pool, res_pool,
                   h1_pool, out_pool, sq_pool, row_pool, bc_pool, mm_psum,
                   a_sb, wv_sb, w1_sb, w2_sb, b1_sb, b2_sb, g_sb, lb_sb,
                   ones_col_b, ones_row, rw_sb, w1bb1_sb,
                   emit_ln_fold_precompute if _iter == 0 else None)

    ctx.close()


def _emit_iter(nc, tc, x_sb, xtm_sb, outT, big, qt_pool, exp_pool, res_pool,
               h1_pool, out_pool, sq_pool, row_pool, bc_pool, mm_psum,
               a_sb, wv_sb, w1_sb, w2_sb, b1_sb, b2_sb, g_sb, lb_sb,
               ones_col_b, ones_row, rw_sb, w1bb1_sb,
               precompute=None):
    # ------- GT = A-stationary GEMM (G = x @ A; scores = G @ x^T) -------
    qt_tiles = []
    for sb in range(NBLK):
        s0 = sb * SBLK
        qt_sb = qt_pool.tile([128, ND, SBLK], BF16, tag="qt")
        for ec in range(ND):
            qps = mm_psum.tile([128, 512], F32, tag="mm")
            for dc in range(ND):
                nc.tensor.matmul(
                    qps[:, :],
                    a_sb[:, dc, ec * 128:(ec + 1) * 128],
                    x_sb[:, dc, s0:s0 + SBLK],
                    start=(dc == 0), stop=(dc == ND - 1),
                )
            nc.scalar.copy(out=qt_sb[:, ec, :], in_=qps[:, :])
        qt_tiles.append(qt_sb)

    # ---------------- per s-block pipeline (software-pipelined emission) ----
    # emission order: scores(0), res(0), scores(1), stats(0), res(1),
    # norm+mlp(0), stats(1), norm+mlp(1) - keeps matmul work queued on PE
    # while DVE/ACT compute the LN row stats of the previous block.
    exp_tiles = [None] * NBLK
    res_tiles = [None] * NBLK
    rows2_tiles = [None] * NBLK

    def emit_scores(sb):
        qt_sb = qt_tiles[sb]
        exp_sb = exp_pool.tile([128, NT, SBLK], BF16, tag="exp", name=f"exp{sb}")
        for tc_i in range(NT):
            sps = mm_psum.tile([128, 512], F32, tag="mm")
            for dc in range(ND):
                nc.tensor.matmul(
                    sps[:, :],
                    x_sb[:, dc, tc_i * 128:(tc_i + 1) * 128],
                    qt_sb[:, dc, :],
                    start=(dc == 0), stop=(dc == ND - 1),
                )
            nc.scalar.activation(out=exp_sb[:, tc_i, :], in_=sps[:, :],
                                 func=AF.Exp, scale=SCALE)
        exp_tiles[sb] = exp_sb

    def emit_res(sb):
        exp_sb = exp_tiles[sb]
        # Z[d, s] = sum_t x[t,d] * exp[t,s]   (x t-major stationary)
        z_sb = sq_pool.tile([128, ND, SBLK], BF16, tag="z", name=f"z{sb}")
        for dc in range(ND):
            zps = mm_psum.tile([128, 512], F32, tag="mm")
            for tc_i in range(NT):
                nc.tensor.matmul(
                    zps[:, :],
                    xtm_sb[:, tc_i, dc * 128:(dc + 1) * 128],
                    exp_sb[:, tc_i, :],
                    start=(tc_i == 0), stop=(tc_i == NT - 1),
                )
            nc.scalar.copy(out=z_sb[:, dc, :], in_=zps[:, :])
        # resU[e, s] = Wv @ Z
        res_sb = res_pool.tile([128, ND, SBLK], BF16, tag="res", name=f"res{sb}")
        for ec in range(ND):
            rps = mm_psum.tile([128, 512], F32, tag="mm")
            for dc in range(ND):
                nc.tensor.matmul(
                    rps[:, :],
                    wv_sb[:, dc, ec * 128:(ec + 1) * 128],
                    z_sb[:, dc, :],
                    start=(dc == 0), stop=(dc == ND - 1),
                )
            nc.vector.tensor_copy(out=res_sb[:, ec, :], in_=rps[:, :])
        res_tiles[sb] = res_sb

    def emit_stats(sb):
        exp_sb = exp_tiles[sb]
        res_sb = res_tiles[sb]
        sums_ps = mm_psum.tile([1, 512], F32, tag="mm")
        for tc_i in range(NT):
            nc.tensor.matmul(
                sums_ps[:, :], ones_col_b[:, :], exp_sb[:, tc_i, :],
                start=(tc_i == 0), stop=(tc_i == NT - 1),
            )
        sume_ps = mm_psum.tile([1, 512], F32, tag="mm")
        for ec in range(ND):
            nc.tensor.matmul(
                sume_ps[:, :], ones_col_b[:, :],
                res_sb[:, ec, :],
                start=(ec == 0), stop=(ec == ND - 1),
            )
        sumsq_ps = mm_psum.tile([1, 512], F32, tag="mm")
        for ec in range(ND):
            sq_sb = sq_pool.tile([128, SBLK], BF16, tag="sq")
            nc.vector.tensor_mul(out=sq_sb[:, :], in0=res_sb[:, ec, :],
                                 in1=res_sb[:, ec, :])
            nc.tensor.matmul(
                sumsq_ps[:, :], ones_col_b[:, :],
                sq_sb[:, :],
                start=(ec == 0), stop=(ec == ND - 1),
            )

        # row stats on one partition:
        #   muU = sumE/512 ; varU = sumSq/512 - muU^2
        #   rstd = 1/sqrt(varU + EPS*sums^2) ; murstd = muU*rstd
        rows = row_pool.tile([1, 4, SBLK], F32, tag="rows", name=f"rows{sb}")
        rows2 = row_pool.tile([1, 2, SBLK], F32R, tag="rows2", name=f"rows2{sb}")
        nc.scalar.mul(out=rows[:, 0, :], in_=sume_ps[:, :], mul=-1.0 / D)    # -muU
        nc.scalar.mul(out=rows[:, 1, :], in_=sumsq_ps[:, :], mul=1.0 / D)    # msq
        nc.scalar.activation(out=rows[:, 2, :], in_=sums_ps[:, :],
                             func=AF.Square, scale=float(np.sqrt(EPS)))      # eps*sums^2
        nc.vector.tensor_mul(out=rows[:, 3, :], in0=rows[:, 0, :], in1=rows[:, 0, :])
        nc.vector.tensor_sub(out=rows[:, 1, :], in0=rows[:, 1, :], in1=rows[:, 3, :])
        nc.vector.tensor_add(out=rows[:, 1, :], in0=rows[:, 1, :], in1=rows[:, 2, :])
        nc.scalar.activation(out=rows[:, 1, :], in_=rows[:, 1, :], func=AF.Sqrt)
        nc.vector.reciprocal_approx_fast(out=rows[:, 3, :], in_=rows[:, 1, :])
        nc.vector.tensor_copy(out=rows2[:, 0, :], in_=rows[:, 3, :])         # rstd
        nc.vector.tensor_mul(out=rows2[:, 1, :], in0=rows[:, 0, :],
                             in1=rows[:, 3, :])                              # -murstd
        rows2_tiles[sb] = rows2

    p_tiles = [None] * NBLK
    h1_tiles = [None] * NBLK

    def emit_p(sb):
        # P = G1 @ res (independent of the LN stats chain)
        res_sb = res_tiles[sb]
        p_ps = []
        for fc in range(ND):
            hps = mm_psum.tile([128, 512], F32, tag="mm", name=f"p{sb}_{fc}")
            for ec in range(ND):
                nc.tensor.matmul(
                    hps[:, :],
                    w1_sb[:, ec, fc * 128:(fc + 1) * 128],
                    res_sb[:, ec, :],
                    start=(ec == 0), stop=(ec == ND - 1),
                )
            p_ps.append(hps)
        p_tiles[sb] = p_ps

    def emit_bc_epi(sb):
        rows2 = rows2_tiles[sb]
        p_ps = p_tiles[sb]

        # broadcast [rstd; -murstd] across 128 partitions via K=1 matmul
        bc_sb = bc_pool.tile([128, 2, SBLK], F32, tag="bc_sb")
        for j in range(2):
            bc_ps = mm_psum.tile([128, 512], F32, tag="mm")
            nc.tensor.matmul(
                bc_ps[:, :], ones_row[:, :],
                rows2[:, j, :], start=True, stop=True,
            )
            nc.scalar.copy(out=bc_sb[:, j, :], in_=bc_ps[:, :])

        # fused MLP1 + LayerNorm epilogue:
        #   h1 = relu( P*rstd[s] - murstd[s]*r1[f] + w1b[f] + b1[f] )
        h1_sb = h1_pool.tile([128, ND, SBLK], BF16, tag="h1", name=f"h1_{sb}")
        for fc in range(ND):
            t_sb = sq_pool.tile([128, SBLK], F32R, tag="sq")
            nc.vector.tensor_mul(out=t_sb[:, :], in0=p_ps[fc][:, :],
                                 in1=bc_sb[:, 0, :])
            nc.vector.scalar_tensor_tensor(
                out=t_sb[:, :], in0=bc_sb[:, 1, :],
                scalar=rw_sb[:, fc, 0:1], in1=t_sb[:, :],
                op0=ALU.mult, op1=ALU.add,
            )
            nc.scalar.activation(out=h1_sb[:, fc, :], in_=t_sb[:, :],
                                 func=AF.Relu, bias=w1bb1_sb[:, fc:fc + 1])
        h1_tiles[sb] = h1_sb

    def emit_mlp2(sb):
        s0 = sb * SBLK
        h1_sb = h1_tiles[sb]
        o_sb = out_pool.tile([128, ND, SBLK], BF16, tag="o")
        outr = outT[:, s0:s0 + SBLK].rearrange("(gc p) s -> p gc s", p=128)
        for gc in range(ND):
            ops = mm_psum.tile([128, 512], F32, tag="mm")
            for fc in range(ND):
                nc.tensor.matmul(
                    ops[:, :],
                    w2_sb[:, fc, gc * 128:(gc + 1) * 128],
                    h1_sb[:, fc, :],
                    start=(fc == 0), stop=(fc == ND - 1),
                )
            nc.scalar.activation(out=o_sb[:, gc, :], in_=ops[:, :],
                                 func=AF.Identity, bias=b2_sb[:, gc:gc + 1])
            eng = (nc.sync, nc.scalar, nc.gpsimd, nc.sync)[gc]
            eng.dma_start(out=outr[:, gc, :], in_=o_sb[:, gc, :])

    emit_scores(0)
    emit_res(0)
    emit_stats(0)
    if precompute is not None:
        precompute()
    emit_scores(1)
    emit_res(1)
    emit_stats(1)
    emit_p(0)
    emit_bc_epi(0)
    emit_p(1)
    emit_bc_epi(1)
    emit_mlp2(0)
    emit_mlp2(1)


def build_nc(n_iters=1):
    nc = bacc.Bacc("TRN2", target_bir_lowering=False, debug=False)
    nc.tensor_by_name = {}

    def dram(name, shape, kind):
        t = nc.dram_tensor(name, shape, F32, kind=kind)
        nc.tensor_by_name[name] = t
        return t

    def dram_bf(name, shape, kind):
        t = nc.dram_tensor(name, shape, BF16, kind=kind)
        nc.tensor_by_name[name] = t
        return t

    dram_bf("xT", [D, S], "ExternalInput")
    dram_bf("xTM", [S, D], "ExternalInput")
    for nm in ("A_qk", "WvT", "W1T", "W2T"):
        dram_bf(nm, [D, D], "ExternalInput")
    for nm in ("b1", "b2", "ln_g", "ln_b"):
        dram(nm, [D], "ExternalInput")
    dram("ones128", [128], "ExternalInput")
    dram_bf("outT", [D, SQ], "ExternalOutput")

    with tile.TileContext(nc) as tc:
        _emit(nc, tc, n_iters=n_iters)
    nc.compile()
    return nc


_CACHED_NC = None


def _get_nc():
    global _CACHED_NC
    if _CACHED_NC is None:
        _CACHED_NC = build_nc()
    return _CACHED_NC


def make_in_maps(x, Wq, Wk, Wv, ln_g, ln_b, W1, b1, W2, b2):
    BF = ml_dtypes.bfloat16
    x = np.asarray(x, dtype=np.float32)
    A_qk = np.asarray(Wq, np.float32).T @ np.asarray(Wk, np.float32)
    shared = {
        "A_qk": np.ascontiguousarray(A_qk.astype(BF)),
        "WvT": np.ascontiguousarray(np.asarray(Wv, np.float32).T.astype(BF)),
        "W1T": np.ascontiguousarray(np.asarray(W1, np.float32).T.astype(BF)),
        "W2T": np.ascontiguousarray(np.asarray(W2, np.float32).T.astype(BF)),
        "b1": np.asarray(b1, np.float32),
        "b2": np.asarray(b2, np.float32),
        "ln_g": np.asarray(ln_g, np.float32),
        "ln_b": np.asarray(ln_b, np.float32),
        "ones128": np.ones(128, np.float32),
    }
    in_maps = []
    for c in range(N_CORES):
        b, h = divmod(c, 2)
        xT = x[:, b, :].T  # (512, 2048)
        q = xT[:, h * SQ:(h + 1) * SQ]
        o = xT[:, (1 - h) * SQ:(2 - h) * SQ]
        xp = np.concatenate([q, o], axis=1)  # (512, 2048), q-half first
        in_maps.append({"xT": np.ascontiguousarray(xp.astype(BF)),
                        "xTM": np.ascontiguousarray(xp.T.astype(BF)),
                        **shared})
    return in_maps


def kernel(x, Wq, Wk, Wv, ln_g, ln_b, W1, b1, W2, b2):
    nc = _get_nc()
    in_maps = make_in_maps(x, Wq, Wk, Wv, ln_g, ln_b, W1, b1, W2, b2)
    res = run_bass_kernel_spmd(nc, in_maps, list(range(N_CORES)))
    out = np.empty((S, B, D), dtype=np.float32)
    for c in range(N_CORES):
        b, h = divmod(c, 2)
        out[h * SQ:(h + 1) * SQ, b, :] = res.results[c]["outT"].T.astype(np.float32)
    return out

